# revision 1
# baseline (speedup 1.0000x reference)
"""Trainium2 Bass kernel for CompositionalGatedRecurrence.

Strategy
--------
8 cores = (batch b, sequence-half s2).  Each core handles ROWS=1024 rows of
one batch with the FULL hidden dim, so RMSNorm and the output projection are
core-local.  The only cross-core coupling is the recurrence state at the
S/2 boundary: a [128, 8] per-pair AllReduce carries the first half's final
state to the second half (double-scan: scan once for the contribution, then
rescan with the received initial state - same SPMD program on every core,
masks select behavior).

Algebra
-------
* top-k primitive selection depends only on the logits -> done on host;
  each bank collapses to a dense W = sum_j w_j * U_j @ V_j, folded on host
  (8.6 GFLOP of numpy, removes 40% of device matmuls).
* log-decay (x @ decay_w + b -> -softplus) is a tiny [8192,1024]@[1024,16]
  matmul -> computed on host in f32; device receives a = sigmoid(-z) = e^ld.
  Protects the scan from bf16 noise.
* rms_w is folded into out_proj_w on host; rstd (per-row scalar) commutes
  with the hid-contraction, so it is applied AFTER the projection as a
  per-partition f32 scale on the [rows, d] psum.
* recurrence state[t] = a[t]*state[t-1] + kv[t] runs as a native
  tensor_tensor_scan (f32 internal) over partitions=hidden lanes,
  free=sequence.
"""

import numpy as np
import ml_dtypes

BF = ml_dtypes.bfloat16

B, S, D = 4, 2048, 1024
H, DH = 16, 64
HID = 1024
NPRIM, RANK = 16, 256
NCORES = 8
ROWS = S // 2          # rows per core
DT = D // 128          # 8 d-model tiles
HT = HID // 128        # 8 hidden tiles
NR = ROWS // 512       # 2 row column-blocks for matmul N
EPS = float(np.finfo(np.float32).eps)

_BUILT = {}


def _build():
    import contextlib
    import concourse.tile as tile
    from concourse import mybir, bacc

    F32 = mybir.dt.float32
    BF16 = mybir.dt.bfloat16
    MULT = mybir.AluOpType.mult
    ADD = mybir.AluOpType.add
    SIG = mybir.ActivationFunctionType.Sigmoid
    SQRT = mybir.ActivationFunctionType.Sqrt
    COPY = mybir.ActivationFunctionType.Copy

    nc = bacc.Bacc()

    # ---- DRAM parameters (per-core shards) --------------------------------
    xt = nc.declare_dram_parameter('xt', [D, ROWS], BF16, isOutput=False)
    bank_w = {}
    for bk in ('g', 'k', 'v', 'q'):
        bank_w[bk] = nc.declare_dram_parameter(f'w{bk}', [D, HID], BF16, isOutput=False)
    ogw = nc.declare_dram_parameter('ogw', [D, HID], BF16, isOutput=False)
    opw = nc.declare_dram_parameter('opw', [HID, D], BF16, isOutput=False)
    a_t = nc.declare_dram_parameter('a_t', [H, ROWS], F32, isOutput=False)
    mc = nc.declare_dram_parameter('mc', [128, 1], F32, isOutput=False)
    ma = nc.declare_dram_parameter('ma', [128, 1], F32, isOutput=False)
    out_d = nc.declare_dram_parameter('out', [ROWS, D], F32, isOutput=True)

    with tile.TileContext(nc, pool_alloc_mode='queue') as tc, \
            contextlib.ExitStack() as ctx:
        p_const = ctx.enter_context(tc.tile_pool(name='const', bufs=1))
        p_kv = ctx.enter_context(tc.tile_pool(name='kvp', bufs=1))
        p_scan = ctx.enter_context(tc.tile_pool(name='scan', bufs=1))
        p_q = ctx.enter_context(tc.tile_pool(name='qp', bufs=1))
        p_ps = ctx.enter_context(tc.tile_pool(name='ps', bufs=8, space='PSUM'))
        p_dram = ctx.enter_context(tc.tile_pool(name='dram', bufs=1, space='DRAM'))

        # ---- whole-kernel residents --------------------------------------
        x_sb = p_const.tile([128, DT, ROWS], BF16)
        nc.sync.dma_start(out=x_sb, in_=xt.rearrange('(t p) r -> p t r', p=128))
        a_sb = p_const.tile([H, ROWS], F32)
        nc.sync.dma_start(out=a_sb, in_=a_t[:, :])
        mc_sb = p_const.tile([128, 1], F32)
        nc.sync.dma_start(out=mc_sb, in_=mc[:, :])
        ma_sb = p_const.tile([128, 1], F32)
        nc.sync.dma_start(out=ma_sb, in_=ma[:, :])
        ones_sb = p_const.tile([128, 1], BF16)
        nc.vector.memset(ones_sb, 1.0)
        st1_last = p_const.tile([128, HT], F32)   # scan1 final cols

        kv_tiles = []
        q_tiles = []

        def build_w(bk, w_sb, p_bank):
            nc.sync.dma_start(
                out=w_sb,
                in_=bank_w[bk].rearrange('(t p) h -> p t h', p=128))

        def y_psum(w_sb, ht, nr, name):
            ps = p_ps.tile([128, 512], F32, tag='ps', name=name)
            for dt in range(DT):
                nc.tensor.matmul(
                    ps,
                    lhsT=w_sb[:, dt, ht * 128:(ht + 1) * 128],
                    rhs=x_sb[:, dt, nr * 512:(nr + 1) * 512],
                    start=(dt == 0), stop=(dt == DT - 1))
            return ps

        def bcast_a(ht, name):
            ab_t = p_scan.tile([128, ROWS], F32, tag='ab', bufs=2, name=name)
            nc.sync.dma_start(
                out=ab_t[0:64, :],
                in_=a_t[2 * ht:2 * ht + 1, :].to_broadcast([64, ROWS]))
            nc.sync.dma_start(
                out=ab_t[64:128, :],
                in_=a_t[2 * ht + 1:2 * ht + 2, :].to_broadcast([64, ROWS]))
            return ab_t

        # =========== phase 1: banks g, k, v with progressive kv fuse ======
        with tc.tile_pool(name='bank', bufs=1) as p_bank:
            with tc.tile_pool(name='fuse', bufs=1) as p_fuse:
                w_g = p_bank.tile([128, DT, HID], BF16, tag='w', bufs=2,
                                  name='w_g')
                build_w('g', w_g, p_bank)
                sigg = []
                for ht in range(HT):
                    sg_t = p_fuse.tile([128, ROWS], BF16, tag='sigg', bufs=8,
                                       name=f'sigg_{ht}')
                    for nr in range(NR):
                        ps = y_psum(w_g, ht, nr, f'yg_{ht}_{nr}')
                        nc.scalar.activation(
                            sg_t[:, nr * 512:(nr + 1) * 512], ps, SIG)
                    sigg.append(sg_t)

                w_k = p_bank.tile([128, DT, HID], BF16, tag='w', bufs=2,
                                  name='w_k')
                build_w('k', w_k, p_bank)
                tgk = []
                for ht in range(HT):
                    tk_t = p_fuse.tile([128, ROWS], BF16, tag='tgk', bufs=8,
                                       name=f'tgk_{ht}')
                    for nr in range(NR):
                        ps = y_psum(w_k, ht, nr, f'yk_{ht}_{nr}')
                        nc.vector.tensor_mul(
                            tk_t[:, nr * 512:(nr + 1) * 512], ps,
                            sigg[ht][:, nr * 512:(nr + 1) * 512])
                    tgk.append(tk_t)

                w_v = p_bank.tile([128, DT, HID], BF16, tag='w', bufs=2,
                                  name='w_v')
                build_w('v', w_v, p_bank)
                for ht in range(HT):
                    kv_t = p_kv.tile([128, ROWS], F32, tag='kv', bufs=8,
                                     name=f'kv_{ht}')
                    for nr in range(NR):
                        ps = y_psum(w_v, ht, nr, f'yv_{ht}_{nr}')
                        nc.vector.tensor_mul(
                            kv_t[:, nr * 512:(nr + 1) * 512], ps,
                            tgk[ht][:, nr * 512:(nr + 1) * 512])
                    kv_tiles.append(kv_t)
                    # scan pass 1 (boundary contribution), init 0
                    ab_t = bcast_a(ht, f'ab1_{ht}')
                    st_t = p_scan.tile([128, ROWS], BF16, tag='st', bufs=3,
                                       name=f'st1_{ht}')
                    nc.vector.tensor_tensor_scan(
                        st_t, ab_t, kv_t, 0.0, MULT, ADD)
                    nc.vector.tensor_copy(st1_last[:, ht:ht + 1],
                                          st_t[:, ROWS - 1:ROWS])
            # p_fuse closed: sigg/tgk freed

            # ---- boundary state exchange (pairs) -------------------------
            contrib = p_const.tile([128, HT], F32)
            nc.vector.tensor_scalar_mul(contrib, st1_last, mc_sb)
            cin = p_dram.tile([128, HT], F32)
            cout = p_dram.tile([128, HT], F32)
            nc.sync.dma_start(out=cin, in_=contrib)
            nc.gpsimd.collective_compute(
                'AllReduce', ADD,
                replica_groups=[[0, 1], [2, 3], [4, 5], [6, 7]],
                ins=[cin.opt()], outs=[cout.opt()])
            s_init = p_const.tile([128, HT], F32)
            nc.sync.dma_start(out=s_init, in_=cout)
            s_eff = p_const.tile([128, HT], F32)
            nc.vector.tensor_scalar_mul(s_eff, s_init, ma_sb)

            # ---- q bank --------------------------------------------------
            w_q = p_bank.tile([128, DT, HID], BF16, tag='w', bufs=2,
                              name='w_q')
            build_w('q', w_q, p_bank)
            for ht in range(HT):
                q_t = p_q.tile([128, ROWS], BF16, tag='q', bufs=8,
                               name=f'q_{ht}')
                for nr in range(NR):
                    ps = y_psum(w_q, ht, nr, f'yq_{ht}_{nr}')
                    nc.vector.tensor_copy(
                        q_t[:, nr * 512:(nr + 1) * 512], ps)
                q_tiles.append(q_t)
        # p_bank closed: u/v/w freed

        # =========== phase 2: post-AR tail ================================
        with tc.tile_pool(name='post', bufs=1) as p_post:
            ogw_sb = p_post.tile([128, DT, HID], BF16)
            nc.sync.dma_start(out=ogw_sb,
                              in_=ogw.rearrange('(t p) h -> p t h', p=128))
            opw_sb = p_post.tile([128, HT, D], BF16)
            nc.sync.dma_start(out=opw_sb,
                              in_=opw.rearrange('(t p) d -> p t d', p=128))

            ss_ps = [p_ps.tile([1, 512], F32, tag='ps', name=f'ss_{nr}')
                     for nr in range(NR)]
            om_tiles = []
            for ht in range(HT):
                ab_t = bcast_a(ht, f'ab2_{ht}')
                st_t = p_scan.tile([128, ROWS], BF16, tag='st', bufs=3,
                                   name=f'st2_{ht}')
                nc.vector.tensor_tensor_scan(
                    st_t, ab_t, kv_tiles[ht], s_eff[:, ht:ht + 1], MULT, ADD)
                out_t = p_scan.tile([128, ROWS], BF16, tag='out', bufs=3,
                                    name=f'out_{ht}')
                nc.vector.tensor_mul(out_t, q_tiles[ht], st_t)
                sq_t = p_scan.tile([128, ROWS], BF16, tag='sq', bufs=2,
                                   name=f'sq_{ht}')
                nc.vector.tensor_mul(sq_t, out_t, out_t)
                for nr in range(NR):
                    nc.tensor.matmul(
                        ss_ps[nr], lhsT=ones_sb,
                        rhs=sq_t[:, nr * 512:(nr + 1) * 512],
                        start=(ht == 0), stop=(ht == HT - 1))
                # out_gate sigmoid, fused into om (pre-norm; rstd applied
                # after the projection)
                om_t = p_post.tile([128, ROWS], BF16, tag='om', bufs=8,
                                   name=f'om_{ht}')
                for nr in range(NR):
                    ps = p_ps.tile([128, 512], F32, tag='ps',
                                   name=f'og_{ht}_{nr}')
                    for dt in range(DT):
                        nc.tensor.matmul(
                            ps,
                            lhsT=ogw_sb[:, dt, ht * 128:(ht + 1) * 128],
                            rhs=x_sb[:, dt, nr * 512:(nr + 1) * 512],
                            start=(dt == 0), stop=(dt == DT - 1))
                    ogs = p_scan.tile([128, 512], BF16, tag='ogs', bufs=2,
                                      name=f'ogs_{ht}_{nr}')
                    nc.scalar.activation(ogs, ps, SIG)
                    nc.vector.tensor_mul(
                        om_t[:, nr * 512:(nr + 1) * 512],
                        out_t[:, nr * 512:(nr + 1) * 512], ogs)
                om_tiles.append(om_t)

            # ---- rstd: sqrt(1/(ms/HID + eps)) as per-row f32 column ------
            rstd = p_const.tile([1, ROWS], F32)
            ms_t = p_const.tile([1, ROWS], F32)
            rec_t = p_const.tile([1, ROWS], F32)
            for nr in range(NR):
                sl = slice(nr * 512, (nr + 1) * 512)
                nc.scalar.activation(ms_t[:, sl], ss_ps[nr], COPY,
                                     scale=1.0 / HID, bias=EPS)
                nc.vector.reciprocal(rec_t[:, sl], ms_t[:, sl])
                nc.scalar.activation(rstd[:, sl], rec_t[:, sl], SQRT)
            # transpose [1, 1024] -> [128, 8] via DRAM bounce
            r_dram = p_dram.tile([1, ROWS], F32)
            nc.sync.dma_start(out=r_dram, in_=rstd)
            rstd_pc = p_const.tile([128, DT], F32)
            nc.sync.dma_start(out=rstd_pc,
                              in_=r_dram.rearrange('one (m p) -> p (one m)', p=128))

            # ---- projection + per-row rstd scale -------------------------
            for nd in range(2):
                pps = []
                for mr in range(DT):
                    pps.append(p_ps.tile([128, 512], F32, tag='ps',
                                         name=f'pj_{nd}_{mr}'))
                for kt in range(HT):
                    for mr in range(DT):
                        nc.tensor.matmul(
                            pps[mr],
                            lhsT=om_tiles[kt][:, mr * 128:(mr + 1) * 128],
                            rhs=opw_sb[:, kt, nd * 512:(nd + 1) * 512],
                            start=(kt == 0), stop=(kt == HT - 1))
                for mr in range(DT):
                    fin_t = p_scan.tile([128, 512], F32, tag='fin', bufs=4,
                                        name=f'fin_{nd}_{mr}')
                    nc.vector.tensor_scalar_mul(fin_t, pps[mr],
                                                rstd_pc[:, mr:mr + 1])
                    nc.sync.dma_start(
                        out=out_d[mr * 128:(mr + 1) * 128,
                                  nd * 512:(nd + 1) * 512],
                        in_=fin_t)

    nc.finalize()
    return nc


def _softmax(x):
    e = np.exp(x - x.max())
    return e / e.sum()


def _host_prep(inputs):
    """Build the 8 per-core input maps."""
    x = np.asarray(inputs['x'], np.float32)
    top_k = int(inputs['top_k'])

    def bank(U, V, logits):
        w = _softmax(np.asarray(logits, np.float32))
        idx = np.argsort(-w)[:top_k]
        vals = w[idx]
        vals = vals / vals.sum()
        U = np.asarray(U, np.float32)[idx]              # [k, D, R]
        V = np.asarray(V, np.float32)[idx]              # [k, R, HID]
        ucat = np.transpose(U, (1, 0, 2)).reshape(D, top_k * RANK)
        vcat = (V * vals[:, None, None]).reshape(top_k * RANK, HID)
        return np.ascontiguousarray(ucat @ vcat).astype(BF)   # [D, HID]

    wg = bank(inputs['v_U'], inputs['v_V'], inputs['gate_logits'])
    wk = bank(inputs['k_U'], inputs['k_V'], inputs['k_logits'])
    wv = bank(inputs['v_U'], inputs['v_V'], inputs['v_logits'])
    wq = bank(inputs['q_U'], inputs['q_V'], inputs['q_logits'])

    ogw = np.ascontiguousarray(
        np.asarray(inputs['out_gate_w'], np.float32).T).astype(BF)   # [D, HID]
    opw = np.ascontiguousarray(
        (np.asarray(inputs['out_proj_w'], np.float32)
         * np.asarray(inputs['rms_w'], np.float32)[None, :]).T).astype(BF)  # [HID, D]

    # decay on host (f32): z = x @ decay_w.T + b ; ld = -softplus(z)
    dw = np.asarray(inputs['decay_w'], np.float32)        # [H, D]
    db = np.asarray(inputs['decay_b'], np.float32)        # [H]
    z = np.einsum('bsd,hd->bsh', x, dw) + db              # [B, S, H]
    a = np.exp(-np.logaddexp(0.0, z))                     # sigmoid(-z) = e^ld

    in_maps = []
    for c in range(NCORES):
        b, s2 = c // 2, c % 2
        sl = slice(s2 * ROWS, (s2 + 1) * ROWS)
        xt = np.ascontiguousarray(x[b, sl].T).astype(BF)           # [D, ROWS]
        a_c = np.ascontiguousarray(a[b, sl].T).astype(np.float32)  # [H, ROWS]
        m_first = 1.0 if s2 == 0 else 0.0
        in_maps.append({
            'xt': xt,
            'wg': wg, 'wk': wk, 'wv': wv, 'wq': wq,
            'ogw': ogw, 'opw': opw,
            'a_t': a_c,
            'mc': np.full((128, 1), m_first, np.float32),
            'ma': np.full((128, 1), 1.0 - m_first, np.float32),
        })
    return in_maps


def kernel(**inputs) -> np.ndarray:
    from concourse.bass_utils import run_bass_kernel_spmd

    if 'nc' not in _BUILT:
        _BUILT['nc'] = _build()
    nc = _BUILT['nc']

    in_maps = _host_prep(inputs)
    res = run_bass_kernel_spmd(nc, in_maps, core_ids=list(range(NCORES)))

    out = np.empty((B, S, D), np.float32)
    for c in range(NCORES):
        b, s2 = c // 2, c % 2
        out[b, s2 * ROWS:(s2 + 1) * ROWS, :] = res.results[c]['out']
    return out



# revision 4
# speedup vs baseline: 1.1836x; 1.1836x over previous
"""Trainium2 Bass kernel for CompositionalGatedRecurrence.

Strategy
--------
8 cores = (batch b, sequence-half s2).  Each core handles ROWS=1024 rows of
one batch with the FULL hidden dim, so RMSNorm and the output projection are
core-local.  The only cross-core coupling is the recurrence state at the
S/2 boundary: a [128, 8] per-pair AllReduce carries the first half's final
state to the second half.

Unlike the double-scan variant, the recurrence runs ONCE per tile with zero
init; the cross-core initial state is folded in afterwards via
    state_full[t] = state_local[t] + cumprod_a[t] * s_init
(one fused scalar_tensor_tensor per hidden tile), so the AllReduce latency
is hidden behind the two matmul banks (out_gate, q) that do not depend on it.

Engine schedule (PE FIFO order): g, k, v banks -> (AllReduce in flight)
og bank -> q bank -> sum-of-squares -> projection.  The PE never waits on
the collective.

Algebra
-------
* top-k primitive selection depends only on the logits -> done on host;
  each bank collapses to a dense W = sum_j w_j * U_j @ V_j, folded on host.
* log-decay computed on host in f32; device receives a = sigmoid(-z) = e^ld
  and its per-row running product cum_a = exp(cumsum(ld)).
* rms_w is folded into out_proj_w on host; rstd (per-row scalar) commutes
  with the hid-contraction, so it is applied AFTER the projection as a
  per-partition f32 scale on the [rows, d] psum.
* all big tensors are pre-arranged on host into [128, tile, free] layouts so
  every load is a max-rate linear DMA; x and the first bank's weights are
  chunked per dt-tile and the first bank accumulates dt-outer across 8 PSUM
  banks, so matmuls start as soon as the first chunks land.
"""

import numpy as np
import ml_dtypes

BF = ml_dtypes.bfloat16

B, S, D = 4, 2048, 1024
H, DH = 16, 64
HID = 1024
NPRIM, RANK = 16, 256
NCORES = 8
ROWS = S // 2          # rows per core
DT = D // 128          # 8 d-model tiles
HT = HID // 128        # 8 hidden tiles
NR = ROWS // 512       # 2 row column-blocks for matmul N
EPS = float(np.finfo(np.float32).eps)
HID_EPS = float(HID * EPS)

_BUILT = {}


def _build():
    import contextlib
    import concourse.tile as tile
    from concourse import mybir, bacc

    F32 = mybir.dt.float32
    BF16 = mybir.dt.bfloat16
    MULT = mybir.AluOpType.mult
    ADD = mybir.AluOpType.add
    SIG = mybir.ActivationFunctionType.Sigmoid
    SQRT = mybir.ActivationFunctionType.Sqrt
    SQUARE = mybir.ActivationFunctionType.Square

    nc = bacc.Bacc()

    # ---- DRAM parameters (per-core shards, host-linearized) ---------------
    xt = nc.declare_dram_parameter('xt', [128, DT, ROWS], BF16, isOutput=False)
    bank_w = {}
    for bk in ('g', 'k', 'v', 'q'):
        bank_w[bk] = nc.declare_dram_parameter(
            f'w{bk}', [128, DT, HID], BF16, isOutput=False)
    ogw = nc.declare_dram_parameter('ogw', [128, DT, HID], BF16, isOutput=False)
    opw = nc.declare_dram_parameter('opw', [128, HT, D], BF16, isOutput=False)
    a_t = nc.declare_dram_parameter('a_t', [H, ROWS], F32, isOutput=False)
    ca_t = nc.declare_dram_parameter('ca_t', [H, ROWS], BF16, isOutput=False)
    mc = nc.declare_dram_parameter('mc', [128, 1], F32, isOutput=False)
    ma = nc.declare_dram_parameter('ma', [128, 1], F32, isOutput=False)
    out_d = nc.declare_dram_parameter('out', [ROWS, D], BF16, isOutput=True)

    with tile.TileContext(nc, pool_alloc_mode='queue') as tc, \
            contextlib.ExitStack() as ctx:
        p_res = ctx.enter_context(tc.tile_pool(name='res', bufs=1))
        p_ps = ctx.enter_context(tc.tile_pool(name='ps', bufs=8, space='PSUM'))
        p_dram = ctx.enter_context(tc.tile_pool(name='dram', bufs=1, space='DRAM'))

        # ---- whole-kernel residents --------------------------------------
        x_sb = p_res.tile([128, DT, ROWS], BF16)
        mc_sb = p_res.tile([128, 1], F32)
        ma_sb = p_res.tile([128, 1], F32)
        ones_sb = p_res.tile([128, 1], BF16)
        st1_last = p_res.tile([128, HT], F32)   # scan final cols
        s_eff = p_res.tile([128, HT], F32)
        opw_sb = p_res.tile([128, HT, D], BF16)
        st_tiles = [p_res.tile([128, ROWS], BF16, name=f'st_{ht}')
                    for ht in range(HT)]

        nc.vector.memset(ones_sb, 1.0)

        def y_psum(w_sb, ht, nr, name):
            """Standard per-(ht,nr) dt-accumulated matmul into one psum bank."""
            ps = p_ps.tile([128, 512], F32, tag='ps', name=name)
            for dt in range(DT):
                nc.tensor.matmul(
                    ps,
                    lhsT=w_sb[:, dt, ht * 128:(ht + 1) * 128],
                    rhs=x_sb[:, dt, nr * 512:(nr + 1) * 512],
                    start=(dt == 0), stop=(dt == DT - 1))
            return ps

        def bcast2(dst, src, ht):
            """Broadcast the two head rows of tile ht to 64 lanes each."""
            nc.sync.dma_start(
                out=dst[0:64, :],
                in_=src[2 * ht:2 * ht + 1, :].to_broadcast([64, ROWS]))
            nc.sync.dma_start(
                out=dst[64:128, :],
                in_=src[2 * ht + 1:2 * ht + 2, :].to_broadcast([64, ROWS]))

        # =========== phase 1: banks g, k, v with progressive kv fuse ======
        with tc.tile_pool(name='bank', bufs=1) as p_bank:
            with tc.tile_pool(name='fuse', bufs=1) as p_fuse, \
                    tc.tile_pool(name='abp', bufs=1) as p_ab:
                # g bank: chunked DMA + dt-outer accumulation for early start
                w_g = p_bank.tile([128, DT, HID], BF16, tag='w', bufs=2,
                                  name='w_g')
                for dt in range(DT):
                    nc.sync.dma_start(out=w_g[:, dt, :],
                                      in_=bank_w['g'][:, dt, :])
                    nc.sync.dma_start(out=x_sb[:, dt, :], in_=xt[:, dt, :])
                nc.sync.dma_start(out=mc_sb, in_=mc[:, :])
                nc.sync.dma_start(out=ma_sb, in_=ma[:, :])

                sigg = [p_fuse.tile([128, ROWS], BF16, tag='sigg', bufs=8,
                                    name=f'sigg_{ht}') for ht in range(HT)]
                for nr in range(NR):
                    pss = [p_ps.tile([128, 512], F32, tag='ps',
                                     name=f'yg_{ht}_{nr}') for ht in range(HT)]
                    for dt in range(DT):
                        for ht in range(HT):
                            nc.tensor.matmul(
                                pss[ht],
                                lhsT=w_g[:, dt, ht * 128:(ht + 1) * 128],
                                rhs=x_sb[:, dt, nr * 512:(nr + 1) * 512],
                                start=(dt == 0), stop=(dt == DT - 1))
                    for ht in range(HT):
                        nc.scalar.activation(
                            sigg[ht][:, nr * 512:(nr + 1) * 512], pss[ht], SIG)

                # k bank
                w_k = p_bank.tile([128, DT, HID], BF16, tag='w', bufs=2,
                                  name='w_k')
                nc.sync.dma_start(out=w_k, in_=bank_w['k'][:, :, :])
                tgk = []
                for ht in range(HT):
                    tk_t = p_fuse.tile([128, ROWS], BF16, tag='tgk', bufs=8,
                                       name=f'tgk_{ht}')
                    for nr in range(NR):
                        sl = slice(nr * 512, (nr + 1) * 512)
                        ps = y_psum(w_k, ht, nr, f'yk_{ht}_{nr}')
                        nc.vector.tensor_mul(tk_t[:, sl], ps, sigg[ht][:, sl])
                    tgk.append(tk_t)

                # v bank + scan (zero-init local states)
                w_v = p_bank.tile([128, DT, HID], BF16, tag='w', bufs=2,
                                  name='w_v')
                nc.sync.dma_start(out=w_v, in_=bank_w['v'][:, :, :])
                # prefetch the projection weights while PE is busy here
                nc.sync.dma_start(out=opw_sb,
                                  in_=opw[:, :, :])
                for ht in range(HT):
                    kv_t = p_fuse.tile([128, ROWS], F32, tag='kv', bufs=3,
                                       name=f'kv_{ht}')
                    for nr in range(NR):
                        sl = slice(nr * 512, (nr + 1) * 512)
                        ps = y_psum(w_v, ht, nr, f'yv_{ht}_{nr}')
                        nc.vector.tensor_mul(kv_t[:, sl], ps, tgk[ht][:, sl])
                    ab_t = p_ab.tile([128, ROWS], F32, tag='ab', bufs=2,
                                     name=f'ab_{ht}')
                    bcast2(ab_t, a_t, ht)
                    nc.vector.tensor_tensor_scan(
                        st_tiles[ht], ab_t, kv_t, 0.0, MULT, ADD)
                    nc.vector.tensor_copy(st1_last[:, ht:ht + 1],
                                          st_tiles[ht][:, ROWS - 1:ROWS])

                # ---- boundary state exchange (pairs) ---------------------
                contrib = p_res.tile([128, HT], F32)
                nc.vector.tensor_scalar_mul(contrib, st1_last, mc_sb)
                cin = p_dram.tile([128, HT], F32)
                cout = p_dram.tile([128, HT], F32)
                nc.sync.dma_start(out=cin, in_=contrib)
                nc.gpsimd.collective_compute(
                    'AllReduce', ADD,
                    replica_groups=[[0, 1], [2, 3], [4, 5], [6, 7]],
                    ins=[cin.opt()], outs=[cout.opt()])
                s_init = p_res.tile([128, HT], F32)
                nc.sync.dma_start(out=s_init, in_=cout)
                nc.vector.tensor_scalar_mul(s_eff, s_init, ma_sb)
            # p_fuse / p_ab closed: sigg, tgk, kv, ab freed

            # =========== phase 2: og bank, correction, q bank =============
            with tc.tile_pool(name='post', bufs=1) as p_post:
                # og bank (AR-independent: hides the collective)
                w_og = p_bank.tile([128, DT, HID], BF16, tag='w', bufs=2,
                                   name='w_og')
                nc.sync.dma_start(out=w_og, in_=ogw[:, :, :])
                ogs = []
                for ht in range(HT):
                    og_t = p_post.tile([128, ROWS], BF16, tag='ogs', bufs=8,
                                       name=f'ogs_{ht}')
                    for nr in range(NR):
                        sl = slice(nr * 512, (nr + 1) * 512)
                        ps = y_psum(w_og, ht, nr, f'yo_{ht}_{nr}')
                        nc.scalar.activation(og_t[:, sl], ps, SIG)
                    ogs.append(og_t)

                # cross-core state correction: stc = cum_a * s_eff + st
                stc = []
                for ht in range(HT):
                    cab_t = p_post.tile([128, ROWS], BF16, tag='cab', bufs=3,
                                        name=f'cab_{ht}')
                    bcast2(cab_t, ca_t, ht)
                    stc_t = p_post.tile([128, ROWS], BF16, tag='stc', bufs=8,
                                        name=f'stc_{ht}')
                    nc.vector.scalar_tensor_tensor(
                        stc_t, cab_t, s_eff[:, ht:ht + 1], st_tiles[ht],
                        MULT, ADD)
                    stc.append(stc_t)

                # q bank, fused out = psum_q * stc, squares + out-gate mul
                w_q = p_bank.tile([128, DT, HID], BF16, tag='w', bufs=2,
                                  name='w_q')
                nc.sync.dma_start(out=w_q, in_=bank_w['q'][:, :, :])
                sq_tiles = []
                om_tiles = []
                for ht in range(HT):
                    pss = [y_psum(w_q, ht, nr, f'yq_{ht}_{nr}')
                           for nr in range(NR)]
                    out_t = p_post.tile([128, ROWS], BF16, tag='out', bufs=8,
                                        name=f'out_{ht}')
                    for nr in range(NR):
                        sl = slice(nr * 512, (nr + 1) * 512)
                        nc.vector.scalar_tensor_tensor(
                            out_t[:, sl], pss[nr], 1.0, stc[ht][:, sl],
                            MULT, MULT)
                    sq_t = p_post.tile([128, ROWS], BF16, tag='sq', bufs=8,
                                       name=f'sq_{ht}')
                    nc.scalar.activation(sq_t, out_t, SQUARE)
                    om_t = p_post.tile([128, ROWS], BF16, tag='om', bufs=8,
                                       name=f'om_{ht}')
                    nc.vector.tensor_mul(om_t, out_t, ogs[ht])
                    sq_tiles.append(sq_t)
                    om_tiles.append(om_t)

                # sum-of-squares (after all q matmuls in the PE FIFO)
                ss_ps = [p_ps.tile([1, 512], F32, tag='ps', name=f'ss_{nr}')
                         for nr in range(NR)]
                for ht in range(HT):
                    for nr in range(NR):
                        nc.tensor.matmul(
                            ss_ps[nr], lhsT=ones_sb,
                            rhs=sq_tiles[ht][:, nr * 512:(nr + 1) * 512],
                            start=(ht == 0), stop=(ht == HT - 1))

                # rstd = sqrt(HID / (ss + HID*eps)), via DRAM-bounce
                # transpose so the reciprocal runs on 128 lanes
                ss_sb = p_res.tile([1, ROWS], F32)
                for nr in range(NR):
                    nc.vector.tensor_copy(
                        ss_sb[:, nr * 512:(nr + 1) * 512], ss_ps[nr])
                r_dram = p_dram.tile([1, ROWS], F32)
                nc.sync.dma_start(out=r_dram, in_=ss_sb)
                ss_pc = p_res.tile([128, DT], F32)
                nc.sync.dma_start(
                    out=ss_pc,
                    in_=r_dram.rearrange('one (m p) -> p (one m)', p=128))
                msb = p_res.tile([128, DT], F32)
                nc.vector.tensor_scalar_add(msb, ss_pc, HID_EPS)
                rec = p_res.tile([128, DT], F32)
                nc.vector.reciprocal(rec, msb)
                rstd_pc = p_res.tile([128, DT], F32)
                nc.scalar.activation(rstd_pc, rec, SQRT, scale=float(HID))

                # ---- projection (mr-outer: low PSUM pressure) ------------
                for nd in range(2):
                    for mr in range(DT):
                        pj = p_ps.tile([128, 512], F32, tag='ps',
                                       name=f'pj_{nd}_{mr}')
                        for kt in range(HT):
                            nc.tensor.matmul(
                                pj,
                                lhsT=om_tiles[kt][:, mr * 128:(mr + 1) * 128],
                                rhs=opw_sb[:, kt, nd * 512:(nd + 1) * 512],
                                start=(kt == 0), stop=(kt == HT - 1))
                        fin_t = p_post.tile([128, 512], BF16, tag='fin',
                                            bufs=4, name=f'fin_{nd}_{mr}')
                        nc.vector.tensor_scalar_mul(fin_t, pj,
                                                    rstd_pc[:, mr:mr + 1])
                        nc.sync.dma_start(
                            out=out_d[mr * 128:(mr + 1) * 128,
                                      nd * 512:(nd + 1) * 512],
                            in_=fin_t)

    nc.finalize()
    return nc


def _softmax(x):
    e = np.exp(x - x.max())
    return e / e.sum()


def _lin128(W, nt):
    """[nt*128, free] f32 -> [128, nt, free] bf16 (partition-major linear)."""
    free = W.shape[1]
    return np.ascontiguousarray(
        W.reshape(nt, 128, free).transpose(1, 0, 2)).astype(BF)


def _host_prep(inputs):
    """Build the 8 per-core input maps."""
    x = np.asarray(inputs['x'], np.float32)
    top_k = int(inputs['top_k'])

    def bank(U, V, logits):
        w = _softmax(np.asarray(logits, np.float32))
        idx = np.argsort(-w)[:top_k]
        vals = w[idx]
        vals = vals / vals.sum()
        U = np.asarray(U, np.float32)[idx]              # [k, D, R]
        V = np.asarray(V, np.float32)[idx]              # [k, R, HID]
        ucat = np.transpose(U, (1, 0, 2)).reshape(D, top_k * RANK)
        vcat = (V * vals[:, None, None]).reshape(top_k * RANK, HID)
        return _lin128(ucat @ vcat, DT)                 # [128, DT, HID]

    wg = bank(inputs['v_U'], inputs['v_V'], inputs['gate_logits'])
    wk = bank(inputs['k_U'], inputs['k_V'], inputs['k_logits'])
    wv = bank(inputs['v_U'], inputs['v_V'], inputs['v_logits'])
    wq = bank(inputs['q_U'], inputs['q_V'], inputs['q_logits'])

    ogw = _lin128(np.ascontiguousarray(
        np.asarray(inputs['out_gate_w'], np.float32).T), DT)      # [128,DT,HID]
    opw = _lin128(np.ascontiguousarray(
        (np.asarray(inputs['out_proj_w'], np.float32)
         * np.asarray(inputs['rms_w'], np.float32)[None, :]).T), HT)  # [128,HT,D]

    # decay on host (f32): z = x @ decay_w.T + b ; ld = -softplus(z)
    dw = np.asarray(inputs['decay_w'], np.float32)        # [H, D]
    db = np.asarray(inputs['decay_b'], np.float32)        # [H]
    z = np.einsum('bsd,hd->bsh', x, dw) + db              # [B, S, H]
    ld = -np.logaddexp(0.0, z)                            # log a
    a = np.exp(ld)                                        # sigmoid(-z)

    in_maps = []
    for c in range(NCORES):
        b, s2 = c // 2, c % 2
        sl = slice(s2 * ROWS, (s2 + 1) * ROWS)
        xt = _lin128(np.ascontiguousarray(x[b, sl].T), DT)          # [128,DT,ROWS]
        a_c = np.ascontiguousarray(a[b, sl].T).astype(np.float32)   # [H, ROWS]
        ca_c = np.ascontiguousarray(
            np.exp(np.cumsum(ld[b, sl], axis=0)).T).astype(BF)      # [H, ROWS]
        m_first = 1.0 if s2 == 0 else 0.0
        in_maps.append({
            'xt': xt,
            'wg': wg, 'wk': wk, 'wv': wv, 'wq': wq,
            'ogw': ogw, 'opw': opw,
            'a_t': a_c, 'ca_t': ca_c,
            'mc': np.full((128, 1), m_first, np.float32),
            'ma': np.full((128, 1), 1.0 - m_first, np.float32),
        })
    return in_maps


def kernel(**inputs) -> np.ndarray:
    from concourse.bass_utils import run_bass_kernel_spmd

    if 'nc' not in _BUILT:
        _BUILT['nc'] = _build()
    nc = _BUILT['nc']

    in_maps = _host_prep(inputs)
    res = run_bass_kernel_spmd(nc, in_maps, core_ids=list(range(NCORES)))

    out = np.empty((B, S, D), np.float32)
    for c in range(NCORES):
        b, s2 = c // 2, c % 2
        out[b, s2 * ROWS:(s2 + 1) * ROWS, :] = \
            np.asarray(res.results[c]['out'], dtype=np.float32)
    return out


# revision 6
# speedup vs baseline: 1.2022x; 1.0157x over previous
"""Trainium2 Bass kernel for CompositionalGatedRecurrence.

Strategy
--------
8 cores = (batch b, sequence-half s2).  Each core handles ROWS=1024 rows of
one batch with the FULL hidden dim, so RMSNorm and the output projection are
core-local.  The only cross-core coupling is the recurrence state at the
S/2 boundary: a [128, 8] per-pair AllReduce carries the first half's final
state to the second half.

Unlike the double-scan variant, the recurrence runs ONCE per tile with zero
init; the cross-core initial state is folded in afterwards via
    state_full[t] = state_local[t] + cumprod_a[t] * s_init
(one fused scalar_tensor_tensor per hidden tile), so the AllReduce latency
is hidden behind the two matmul banks (out_gate, q) that do not depend on it.

Engine schedule (PE FIFO order): g, k, v banks -> (AllReduce in flight)
og bank -> q bank -> sum-of-squares -> projection.  The PE never waits on
the collective.

Algebra
-------
* top-k primitive selection depends only on the logits -> done on host;
  each bank collapses to a dense W = sum_j w_j * U_j @ V_j, folded on host.
* log-decay computed on host in f32; device receives a = sigmoid(-z) = e^ld
  and its per-row running product cum_a = exp(cumsum(ld)).
* rms_w is folded into out_proj_w on host; rstd (per-row scalar) commutes
  with the hid-contraction, so it is applied AFTER the projection as a
  per-partition f32 scale on the [rows, d] psum.
* all big tensors are pre-arranged on host into [128, tile, free] layouts so
  every load is a max-rate linear DMA; x and the first bank's weights are
  chunked per dt-tile and the first bank accumulates dt-outer across 8 PSUM
  banks, so matmuls start as soon as the first chunks land.
"""

import numpy as np
import ml_dtypes

BF = ml_dtypes.bfloat16

B, S, D = 4, 2048, 1024
H, DH = 16, 64
HID = 1024
NPRIM, RANK = 16, 256
NCORES = 8
ROWS = S // 2          # rows per core
DT = D // 128          # 8 d-model tiles
HT = HID // 128        # 8 hidden tiles
NR = ROWS // 512       # 2 row column-blocks for matmul N
EPS = float(np.finfo(np.float32).eps)
HID_EPS = float(HID * EPS)

_BUILT = {}


def _build():
    import contextlib
    import concourse.tile as tile
    from concourse import mybir, bacc

    F32 = mybir.dt.float32
    BF16 = mybir.dt.bfloat16
    MULT = mybir.AluOpType.mult
    ADD = mybir.AluOpType.add
    SIG = mybir.ActivationFunctionType.Sigmoid
    SQRT = mybir.ActivationFunctionType.Sqrt
    SQUARE = mybir.ActivationFunctionType.Square
    COPY = mybir.ActivationFunctionType.Copy

    nc = bacc.Bacc()

    # ---- DRAM parameters (per-core shards, host-linearized) ---------------
    xt = nc.declare_dram_parameter('xt', [128, DT, ROWS], BF16, isOutput=False)
    bank_w = {}
    for bk in ('g', 'k', 'v', 'q'):
        bank_w[bk] = nc.declare_dram_parameter(
            f'w{bk}', [128, DT, HID], BF16, isOutput=False)
    ogw = nc.declare_dram_parameter('ogw', [128, DT, HID], BF16, isOutput=False)
    opw = nc.declare_dram_parameter('opw', [128, HT, D], BF16, isOutput=False)
    a_t = nc.declare_dram_parameter('a_t', [H, ROWS], F32, isOutput=False)
    ca_t = nc.declare_dram_parameter('ca_t', [H, ROWS], BF16, isOutput=False)
    mc = nc.declare_dram_parameter('mc', [128, 1], F32, isOutput=False)
    ma = nc.declare_dram_parameter('ma', [128, 1], F32, isOutput=False)
    out_d = nc.declare_dram_parameter('out', [ROWS, D], BF16, isOutput=True)

    with tile.TileContext(nc, pool_alloc_mode='queue') as tc, \
            contextlib.ExitStack() as ctx:
        p_res = ctx.enter_context(tc.tile_pool(name='res', bufs=1))
        p_ps = ctx.enter_context(tc.tile_pool(name='ps', bufs=8, space='PSUM'))
        p_dram = ctx.enter_context(tc.tile_pool(name='dram', bufs=1, space='DRAM'))

        # ---- whole-kernel residents --------------------------------------
        x_sb = p_res.tile([128, DT, ROWS], BF16)
        mc_sb = p_res.tile([128, 1], F32)
        ma_sb = p_res.tile([128, 1], F32)
        ones_sb = p_res.tile([128, 1], BF16)
        st1_last = p_res.tile([128, HT], F32)   # scan final cols
        s_eff = p_res.tile([128, HT], F32)
        opw_sb = p_res.tile([128, HT, D], BF16)
        st_tiles = [p_res.tile([128, ROWS], BF16, name=f'st_{ht}')
                    for ht in range(HT)]

        nc.vector.memset(ones_sb, 1.0)

        def y_psum(w_sb, ht, nr, name):
            """Standard per-(ht,nr) dt-accumulated matmul into one psum bank."""
            ps = p_ps.tile([128, 512], F32, tag='ps', name=name)
            for dt in range(DT):
                nc.tensor.matmul(
                    ps,
                    lhsT=w_sb[:, dt, ht * 128:(ht + 1) * 128],
                    rhs=x_sb[:, dt, nr * 512:(nr + 1) * 512],
                    start=(dt == 0), stop=(dt == DT - 1))
            return ps

        def bcast2(dst, src, ht):
            """Broadcast the two head rows of tile ht to 64 lanes each."""
            nc.sync.dma_start(
                out=dst[0:64, :],
                in_=src[2 * ht:2 * ht + 1, :].to_broadcast([64, ROWS]))
            nc.sync.dma_start(
                out=dst[64:128, :],
                in_=src[2 * ht + 1:2 * ht + 2, :].to_broadcast([64, ROWS]))

        # =========== phase 1: banks g, k, v with progressive kv fuse ======
        with tc.tile_pool(name='bank', bufs=1) as p_bank:
            with tc.tile_pool(name='fuse', bufs=1) as p_fuse, \
                    tc.tile_pool(name='abp', bufs=1) as p_ab:
                # g bank: chunked DMA + dt-outer accumulation for early start
                w_g = p_bank.tile([128, DT, HID], BF16, tag='w', bufs=2,
                                  name='w_g')
                for dt in range(DT):
                    nc.sync.dma_start(out=w_g[:, dt, :],
                                      in_=bank_w['g'][:, dt, :])
                    nc.sync.dma_start(out=x_sb[:, dt, :], in_=xt[:, dt, :])
                nc.sync.dma_start(out=mc_sb, in_=mc[:, :])
                nc.sync.dma_start(out=ma_sb, in_=ma[:, :])

                sigg = [p_fuse.tile([128, ROWS], BF16, tag='sigg', bufs=8,
                                    name=f'sigg_{ht}') for ht in range(HT)]
                for nr in range(NR):
                    pss = [p_ps.tile([128, 512], F32, tag='ps',
                                     name=f'yg_{ht}_{nr}') for ht in range(HT)]
                    for dt in range(DT):
                        for ht in range(HT):
                            nc.tensor.matmul(
                                pss[ht],
                                lhsT=w_g[:, dt, ht * 128:(ht + 1) * 128],
                                rhs=x_sb[:, dt, nr * 512:(nr + 1) * 512],
                                start=(dt == 0), stop=(dt == DT - 1))
                    for ht in range(HT):
                        nc.scalar.activation(
                            sigg[ht][:, nr * 512:(nr + 1) * 512], pss[ht], SIG)

                # k bank
                w_k = p_bank.tile([128, DT, HID], BF16, tag='w', bufs=2,
                                  name='w_k')
                nc.sync.dma_start(out=w_k, in_=bank_w['k'][:, :, :])
                tgk = []
                for ht in range(HT):
                    tk_t = p_fuse.tile([128, ROWS], BF16, tag='tgk', bufs=8,
                                       name=f'tgk_{ht}')
                    for nr in range(NR):
                        sl = slice(nr * 512, (nr + 1) * 512)
                        ps = y_psum(w_k, ht, nr, f'yk_{ht}_{nr}')
                        nc.vector.tensor_mul(tk_t[:, sl], ps, sigg[ht][:, sl])
                    tgk.append(tk_t)

                # v bank + scan (zero-init local states)
                w_v = p_bank.tile([128, DT, HID], BF16, tag='w', bufs=2,
                                  name='w_v')
                nc.sync.dma_start(out=w_v, in_=bank_w['v'][:, :, :])
                # prefetch the projection weights while PE is busy here
                nc.sync.dma_start(out=opw_sb,
                                  in_=opw[:, :, :])
                for ht in range(HT):
                    kv_t = p_fuse.tile([128, ROWS], F32, tag='kv', bufs=3,
                                       name=f'kv_{ht}')
                    for nr in range(NR):
                        sl = slice(nr * 512, (nr + 1) * 512)
                        ps = y_psum(w_v, ht, nr, f'yv_{ht}_{nr}')
                        nc.vector.tensor_mul(kv_t[:, sl], ps, tgk[ht][:, sl])
                    ab_t = p_ab.tile([128, ROWS], F32, tag='ab', bufs=2,
                                     name=f'ab_{ht}')
                    bcast2(ab_t, a_t, ht)
                    nc.vector.tensor_tensor_scan(
                        st_tiles[ht], ab_t, kv_t, 0.0, MULT, ADD)
                    nc.vector.tensor_copy(st1_last[:, ht:ht + 1],
                                          st_tiles[ht][:, ROWS - 1:ROWS])

                # ---- boundary state exchange (pairs) ---------------------
                contrib = p_res.tile([128, HT], F32)
                nc.vector.tensor_scalar_mul(contrib, st1_last, mc_sb)
                cin = p_dram.tile([128, HT], F32)
                cout = p_dram.tile([128, HT], F32)
                nc.sync.dma_start(out=cin, in_=contrib)
                nc.gpsimd.collective_compute(
                    'AllReduce', ADD,
                    replica_groups=[[0, 1], [2, 3], [4, 5], [6, 7]],
                    ins=[cin.opt()], outs=[cout.opt()])
                s_init = p_res.tile([128, HT], F32)
                nc.sync.dma_start(out=s_init, in_=cout)
                nc.vector.tensor_scalar_mul(s_eff, s_init, ma_sb)
            # p_fuse / p_ab closed: sigg, tgk, kv, ab freed

            # =========== phase 2: og bank, correction, q bank =============
            with tc.tile_pool(name='post', bufs=1) as p_post:
                # og bank (AR-independent: hides the collective)
                w_og = p_bank.tile([128, DT, HID], BF16, tag='w', bufs=2,
                                   name='w_og')
                nc.sync.dma_start(out=w_og, in_=ogw[:, :, :])
                ogs = []
                for ht in range(HT):
                    og_t = p_post.tile([128, ROWS], BF16, tag='ogs', bufs=8,
                                       name=f'ogs_{ht}')
                    for nr in range(NR):
                        sl = slice(nr * 512, (nr + 1) * 512)
                        ps = y_psum(w_og, ht, nr, f'yo_{ht}_{nr}')
                        nc.scalar.activation(og_t[:, sl], ps, SIG)
                    ogs.append(og_t)

                # cross-core state correction: stc = cum_a * s_eff + st
                # (queued on DVE behind the AllReduce; og/q PE work and the
                # ACT q-copies below do not depend on it)
                stc = []
                for ht in range(HT):
                    cab_t = p_post.tile([128, ROWS], BF16, tag='cab', bufs=3,
                                        name=f'cab_{ht}')
                    bcast2(cab_t, ca_t, ht)
                    stc_t = p_post.tile([128, ROWS], BF16, tag='stc', bufs=8,
                                        name=f'stc_{ht}')
                    nc.vector.scalar_tensor_tensor(
                        stc_t, cab_t, s_eff[:, ht:ht + 1], st_tiles[ht],
                        MULT, ADD)
                    stc.append(stc_t)

                # q bank: psums staged to SBUF via ACT copies (AR-independent
                # so the PE never backpressures on the collective)
                w_q = p_bank.tile([128, DT, HID], BF16, tag='w', bufs=2,
                                  name='w_q')
                nc.sync.dma_start(out=w_q, in_=bank_w['q'][:, :, :])
                q_sb = []
                for ht in range(HT):
                    q_t = p_post.tile([128, ROWS], BF16, tag='q', bufs=8,
                                      name=f'q_{ht}')
                    for nr in range(NR):
                        sl = slice(nr * 512, (nr + 1) * 512)
                        ps = y_psum(w_q, ht, nr, f'yq_{ht}_{nr}')
                        nc.scalar.activation(q_t[:, sl], ps, COPY)
                    q_sb.append(q_t)

                # out = q * stc (bf16 2x), sq = out^2 (ACT), om = out * ogs
                sq_tiles = []
                om_tiles = []
                for ht in range(HT):
                    out_t = p_post.tile([128, ROWS], BF16, tag='out', bufs=8,
                                        name=f'out_{ht}')
                    nc.vector.tensor_mul(out_t, q_sb[ht], stc[ht])
                    sq_t = p_post.tile([128, ROWS], BF16, tag='sq', bufs=8,
                                       name=f'sq_{ht}')
                    nc.scalar.activation(sq_t, out_t, SQUARE)
                    om_t = p_post.tile([128, ROWS], BF16, tag='om', bufs=8,
                                       name=f'om_{ht}')
                    nc.vector.tensor_mul(om_t, out_t, ogs[ht])
                    sq_tiles.append(sq_t)
                    om_tiles.append(om_t)

                # ---- projection nd=0 wave A: kt-outer rounds paced by om
                # production (6 banks) with sum-of-squares interleaved (2)
                WA = 6
                pjA = [p_ps.tile([128, 512], F32, tag='ps', name=f'pjA_{mr}')
                       for mr in range(WA)]
                ss_ps = [p_ps.tile([1, 512], F32, tag='ps', name=f'ss_{nr}')
                         for nr in range(NR)]
                for kt in range(HT):
                    for mr in range(WA):
                        nc.tensor.matmul(
                            pjA[mr],
                            lhsT=om_tiles[kt][:, mr * 128:(mr + 1) * 128],
                            rhs=opw_sb[:, kt, 0:512],
                            start=(kt == 0), stop=(kt == HT - 1))
                    for nr in range(NR):
                        nc.tensor.matmul(
                            ss_ps[nr], lhsT=ones_sb,
                            rhs=sq_tiles[kt][:, nr * 512:(nr + 1) * 512],
                            start=(kt == 0), stop=(kt == HT - 1))

                # rstd = sqrt(HID / (ss + HID*eps)), via DRAM-bounce
                # transpose so the reciprocal runs on 128 lanes
                ss_sb = p_res.tile([1, ROWS], F32)
                for nr in range(NR):
                    nc.vector.tensor_copy(
                        ss_sb[:, nr * 512:(nr + 1) * 512], ss_ps[nr])
                r_dram = p_dram.tile([1, ROWS], F32)
                nc.sync.dma_start(out=r_dram, in_=ss_sb)
                ss_pc = p_res.tile([128, DT], F32)
                nc.sync.dma_start(
                    out=ss_pc,
                    in_=r_dram.rearrange('one (m p) -> p (one m)', p=128))
                msb = p_res.tile([128, DT], F32)
                nc.vector.tensor_scalar_add(msb, ss_pc, HID_EPS)
                rec = p_res.tile([128, DT], F32)
                nc.vector.reciprocal(rec, msb)
                rstd_pc = p_res.tile([128, DT], F32)
                nc.scalar.activation(rstd_pc, rec, SQRT, scale=float(HID))

                def fin_store(pj, nd, mr):
                    fin_t = p_post.tile([128, 512], BF16, tag='fin',
                                        bufs=4, name=f'fin_{nd}_{mr}')
                    nc.vector.tensor_scalar_mul(fin_t, pj,
                                                rstd_pc[:, mr:mr + 1])
                    nc.sync.dma_start(
                        out=out_d[mr * 128:(mr + 1) * 128,
                                  nd * 512:(nd + 1) * 512],
                        in_=fin_t)

                # wave B: the remaining nd=0 row blocks (banks freed by ss)
                for mr in range(WA, DT):
                    pj = p_ps.tile([128, 512], F32, tag='ps', name=f'pjB_{mr}')
                    for kt in range(HT):
                        nc.tensor.matmul(
                            pj,
                            lhsT=om_tiles[kt][:, mr * 128:(mr + 1) * 128],
                            rhs=opw_sb[:, kt, 0:512],
                            start=(kt == 0), stop=(kt == HT - 1))
                    fin_store(pj, 0, mr)
                for mr in range(WA):
                    fin_store(pjA[mr], 0, mr)

                # nd=1: mr-outer rotation (starts as wave-A banks drain)
                for mr in range(DT):
                    pj = p_ps.tile([128, 512], F32, tag='ps', name=f'pj1_{mr}')
                    for kt in range(HT):
                        nc.tensor.matmul(
                            pj,
                            lhsT=om_tiles[kt][:, mr * 128:(mr + 1) * 128],
                            rhs=opw_sb[:, kt, 512:1024],
                            start=(kt == 0), stop=(kt == HT - 1))
                    fin_store(pj, 1, mr)

    nc.finalize()
    return nc


def _softmax(x):
    e = np.exp(x - x.max())
    return e / e.sum()


def _lin128(W, nt):
    """[nt*128, free] f32 -> [128, nt, free] bf16 (partition-major linear)."""
    free = W.shape[1]
    return np.ascontiguousarray(
        W.reshape(nt, 128, free).transpose(1, 0, 2)).astype(BF)


def _host_prep(inputs):
    """Build the 8 per-core input maps."""
    x = np.asarray(inputs['x'], np.float32)
    top_k = int(inputs['top_k'])

    def bank(U, V, logits):
        w = _softmax(np.asarray(logits, np.float32))
        idx = np.argsort(-w)[:top_k]
        vals = w[idx]
        vals = vals / vals.sum()
        U = np.asarray(U, np.float32)[idx]              # [k, D, R]
        V = np.asarray(V, np.float32)[idx]              # [k, R, HID]
        ucat = np.transpose(U, (1, 0, 2)).reshape(D, top_k * RANK)
        vcat = (V * vals[:, None, None]).reshape(top_k * RANK, HID)
        return _lin128(ucat @ vcat, DT)                 # [128, DT, HID]

    wg = bank(inputs['v_U'], inputs['v_V'], inputs['gate_logits'])
    wk = bank(inputs['k_U'], inputs['k_V'], inputs['k_logits'])
    wv = bank(inputs['v_U'], inputs['v_V'], inputs['v_logits'])
    wq = bank(inputs['q_U'], inputs['q_V'], inputs['q_logits'])

    ogw = _lin128(np.ascontiguousarray(
        np.asarray(inputs['out_gate_w'], np.float32).T), DT)      # [128,DT,HID]
    opw = _lin128(np.ascontiguousarray(
        (np.asarray(inputs['out_proj_w'], np.float32)
         * np.asarray(inputs['rms_w'], np.float32)[None, :]).T), HT)  # [128,HT,D]

    # decay on host (f32): z = x @ decay_w.T + b ; ld = -softplus(z)
    dw = np.asarray(inputs['decay_w'], np.float32)        # [H, D]
    db = np.asarray(inputs['decay_b'], np.float32)        # [H]
    z = np.einsum('bsd,hd->bsh', x, dw) + db              # [B, S, H]
    ld = -np.logaddexp(0.0, z)                            # log a
    a = np.exp(ld)                                        # sigmoid(-z)

    in_maps = []
    for c in range(NCORES):
        b, s2 = c // 2, c % 2
        sl = slice(s2 * ROWS, (s2 + 1) * ROWS)
        xt = _lin128(np.ascontiguousarray(x[b, sl].T), DT)          # [128,DT,ROWS]
        a_c = np.ascontiguousarray(a[b, sl].T).astype(np.float32)   # [H, ROWS]
        ca_c = np.ascontiguousarray(
            np.exp(np.cumsum(ld[b, sl], axis=0)).T).astype(BF)      # [H, ROWS]
        m_first = 1.0 if s2 == 0 else 0.0
        in_maps.append({
            'xt': xt,
            'wg': wg, 'wk': wk, 'wv': wv, 'wq': wq,
            'ogw': ogw, 'opw': opw,
            'a_t': a_c, 'ca_t': ca_c,
            'mc': np.full((128, 1), m_first, np.float32),
            'ma': np.full((128, 1), 1.0 - m_first, np.float32),
        })
    return in_maps


def kernel(**inputs) -> np.ndarray:
    from concourse.bass_utils import run_bass_kernel_spmd

    if 'nc' not in _BUILT:
        _BUILT['nc'] = _build()
    nc = _BUILT['nc']

    in_maps = _host_prep(inputs)
    res = run_bass_kernel_spmd(nc, in_maps, core_ids=list(range(NCORES)))

    out = np.empty((B, S, D), np.float32)
    for c in range(NCORES):
        b, s2 = c // 2, c % 2
        out[b, s2 * ROWS:(s2 + 1) * ROWS, :] = \
            np.asarray(res.results[c]['out'], dtype=np.float32)
    return out


# revision 10
# speedup vs baseline: 1.2296x; 1.0228x over previous
"""Trainium2 Bass kernel for CompositionalGatedRecurrence.

Strategy
--------
8 cores = (batch b, sequence-half s2).  Each core handles ROWS=1024 rows of
one batch with the FULL hidden dim, so RMSNorm and the output projection are
core-local.  The only cross-core coupling is the recurrence state at the
S/2 boundary: a [128, 8] per-pair AllReduce carries the first half's final
state to the second half.

Unlike the double-scan variant, the recurrence runs ONCE per tile with zero
init; the cross-core initial state is folded in afterwards via
    state_full[t] = state_local[t] + cumprod_a[t] * s_init
(one fused scalar_tensor_tensor per hidden tile), so the AllReduce latency
is hidden behind the two matmul banks (out_gate, q) that do not depend on it.

Engine schedule (PE FIFO order): g, k, v banks -> (AllReduce in flight)
og bank -> q bank -> sum-of-squares -> projection.  The PE never waits on
the collective.

Algebra
-------
* top-k primitive selection depends only on the logits -> done on host;
  each bank collapses to a dense W = sum_j w_j * U_j @ V_j, folded on host.
* log-decay computed on host in f32; device receives a = sigmoid(-z) = e^ld
  and its per-row running product cum_a = exp(cumsum(ld)).
* rms_w is folded into out_proj_w on host; rstd (per-row scalar) commutes
  with the hid-contraction, so it is applied AFTER the projection as a
  per-partition f32 scale on the [rows, d] psum.
* all big tensors are pre-arranged on host into [128, tile, free] layouts so
  every load is a max-rate linear DMA; x and the first bank's weights are
  chunked per dt-tile and the first bank accumulates dt-outer across 8 PSUM
  banks, so matmuls start as soon as the first chunks land.
"""

import numpy as np
import ml_dtypes

BF = ml_dtypes.bfloat16

B, S, D = 4, 2048, 1024
H, DH = 16, 64
HID = 1024
NPRIM, RANK = 16, 256
NCORES = 8
ROWS = S // 2          # rows per core
DT = D // 128          # 8 d-model tiles
HT = HID // 128        # 8 hidden tiles
NR = ROWS // 512       # 2 row column-blocks for matmul N
EPS = float(np.finfo(np.float32).eps)
HID_EPS = float(HID * EPS)

_BUILT = {}


def _build():
    import contextlib
    import concourse.tile as tile
    from concourse import mybir, bacc

    F32 = mybir.dt.float32
    BF16 = mybir.dt.bfloat16
    MULT = mybir.AluOpType.mult
    ADD = mybir.AluOpType.add
    SIG = mybir.ActivationFunctionType.Sigmoid
    SQRT = mybir.ActivationFunctionType.Sqrt
    SQUARE = mybir.ActivationFunctionType.Square
    COPY = mybir.ActivationFunctionType.Copy

    nc = bacc.Bacc()

    # ---- DRAM parameters (per-core shards, host-linearized) ---------------
    xt = nc.declare_dram_parameter('xt', [128, DT, ROWS], BF16, isOutput=False)
    bank_w = {}
    for bk in ('g', 'k', 'v', 'q'):
        bank_w[bk] = nc.declare_dram_parameter(
            f'w{bk}', [128, DT, HID], BF16, isOutput=False)
    ogw = nc.declare_dram_parameter('ogw', [128, DT, HID], BF16, isOutput=False)
    opw = nc.declare_dram_parameter('opw', [128, HT, D], BF16, isOutput=False)
    a_t = nc.declare_dram_parameter('a_t', [H, ROWS], F32, isOutput=False)
    ca_t = nc.declare_dram_parameter('ca_t', [H, ROWS], BF16, isOutput=False)
    mc = nc.declare_dram_parameter('mc', [128, 1], F32, isOutput=False)
    ma = nc.declare_dram_parameter('ma', [128, 1], F32, isOutput=False)
    out_d = nc.declare_dram_parameter('out', [ROWS, D], BF16, isOutput=True)

    with tile.TileContext(nc, pool_alloc_mode='queue') as tc, \
            contextlib.ExitStack() as ctx:
        p_res = ctx.enter_context(tc.tile_pool(name='res', bufs=1))
        p_ps = ctx.enter_context(tc.tile_pool(name='ps', bufs=8, space='PSUM'))
        p_dram = ctx.enter_context(tc.tile_pool(name='dram', bufs=1, space='DRAM'))

        # ---- whole-kernel residents --------------------------------------
        x_sb = p_res.tile([128, DT, ROWS], BF16)
        mc_sb = p_res.tile([128, 1], F32)
        ma_sb = p_res.tile([128, 1], F32)
        ones_sb = p_res.tile([128, 1], BF16)
        st1_last = p_res.tile([128, HT], F32)   # scan final cols
        s_eff = p_res.tile([128, HT], F32)
        opw_sb = p_res.tile([128, HT, D], BF16)
        st_tiles = [p_res.tile([128, ROWS], BF16, name=f'st_{ht}')
                    for ht in range(HT)]

        nc.vector.memset(ones_sb, 1.0)

        def y_psum(w_sb, ht, nr, name):
            """Standard per-(ht,nr) dt-accumulated matmul into one psum bank."""
            ps = p_ps.tile([128, 512], F32, tag='ps', name=name)
            for dt in range(DT):
                nc.tensor.matmul(
                    ps,
                    lhsT=w_sb[:, dt, ht * 128:(ht + 1) * 128],
                    rhs=x_sb[:, dt, nr * 512:(nr + 1) * 512],
                    start=(dt == 0), stop=(dt == DT - 1))
            return ps

        def bcast2(dst, src, ht):
            """Broadcast the two head rows of tile ht to 64 lanes each."""
            nc.sync.dma_start(
                out=dst[0:64, :],
                in_=src[2 * ht:2 * ht + 1, :].to_broadcast([64, ROWS]))
            nc.sync.dma_start(
                out=dst[64:128, :],
                in_=src[2 * ht + 1:2 * ht + 2, :].to_broadcast([64, ROWS]))

        # =========== phase 1: banks g, k, v with progressive kv fuse ======
        with tc.tile_pool(name='bank', bufs=1) as p_bank:
            with tc.tile_pool(name='fuse', bufs=1) as p_fuse, \
                    tc.tile_pool(name='abp', bufs=1) as p_ab:
                # g bank: chunked DMA + dt-outer accumulation for early start
                w_g = p_bank.tile([128, DT, HID], BF16, tag='w', bufs=2,
                                  name='w_g')
                for dt in range(DT):
                    nc.sync.dma_start(out=w_g[:, dt, :],
                                      in_=bank_w['g'][:, dt, :])
                    nc.sync.dma_start(out=x_sb[:, dt, :], in_=xt[:, dt, :])
                nc.sync.dma_start(out=mc_sb, in_=mc[:, :])
                nc.sync.dma_start(out=ma_sb, in_=ma[:, :])

                sigg = [p_fuse.tile([128, ROWS], BF16, tag='sigg', bufs=8,
                                    name=f'sigg_{ht}') for ht in range(HT)]
                for nr in range(NR):
                    pss = [p_ps.tile([128, 512], F32, tag='ps',
                                     name=f'yg_{ht}_{nr}') for ht in range(HT)]
                    for dt in range(DT):
                        for ht in range(HT):
                            nc.tensor.matmul(
                                pss[ht],
                                lhsT=w_g[:, dt, ht * 128:(ht + 1) * 128],
                                rhs=x_sb[:, dt, nr * 512:(nr + 1) * 512],
                                start=(dt == 0), stop=(dt == DT - 1))
                    for ht in range(HT):
                        nc.scalar.activation(
                            sigg[ht][:, nr * 512:(nr + 1) * 512], pss[ht], SIG)

                # k bank
                w_k = p_bank.tile([128, DT, HID], BF16, tag='w', bufs=2,
                                  name='w_k')
                nc.sync.dma_start(out=w_k, in_=bank_w['k'][:, :, :])
                tgk = []
                for ht in range(HT):
                    tk_t = p_fuse.tile([128, ROWS], BF16, tag='tgk', bufs=8,
                                       name=f'tgk_{ht}')
                    for nr in range(NR):
                        sl = slice(nr * 512, (nr + 1) * 512)
                        ps = y_psum(w_k, ht, nr, f'yk_{ht}_{nr}')
                        nc.vector.tensor_mul(tk_t[:, sl], ps, sigg[ht][:, sl])
                    tgk.append(tk_t)

                # v bank + scan (zero-init local states)
                w_v = p_bank.tile([128, DT, HID], BF16, tag='w', bufs=2,
                                  name='w_v')
                nc.sync.dma_start(out=w_v, in_=bank_w['v'][:, :, :])
                # prefetch the projection weights while PE is busy here
                nc.sync.dma_start(out=opw_sb,
                                  in_=opw[:, :, :])
                for ht in range(HT):
                    kv_t = p_fuse.tile([128, ROWS], F32, tag='kv', bufs=3,
                                       name=f'kv_{ht}')
                    for nr in range(NR):
                        sl = slice(nr * 512, (nr + 1) * 512)
                        ps = y_psum(w_v, ht, nr, f'yv_{ht}_{nr}')
                        nc.vector.tensor_mul(kv_t[:, sl], ps, tgk[ht][:, sl])
                    ab_t = p_ab.tile([128, ROWS], F32, tag='ab', bufs=2,
                                     name=f'ab_{ht}')
                    bcast2(ab_t, a_t, ht)
                    nc.vector.tensor_tensor_scan(
                        st_tiles[ht], ab_t, kv_t, 0.0, MULT, ADD)
                    nc.vector.tensor_copy(st1_last[:, ht:ht + 1],
                                          st_tiles[ht][:, ROWS - 1:ROWS])

                # ---- boundary state exchange (pairs) ---------------------
                contrib = p_res.tile([128, HT], F32)
                nc.vector.tensor_scalar_mul(contrib, st1_last, mc_sb)
                cin = p_dram.tile([128, HT], F32)
                cout = p_dram.tile([128, HT], F32)
                nc.sync.dma_start(out=cin, in_=contrib)
                nc.gpsimd.collective_compute(
                    'AllReduce', ADD,
                    replica_groups=[[0, 1], [2, 3], [4, 5], [6, 7]],
                    ins=[cin.opt()], outs=[cout.opt()])
                s_init = p_res.tile([128, HT], F32)
                nc.sync.dma_start(out=s_init, in_=cout)
                nc.vector.tensor_scalar_mul(s_eff, s_init, ma_sb)
            # p_fuse / p_ab closed: sigg, tgk, kv, ab freed

            # =========== phase 2: og bank, correction, q bank =============
            with tc.tile_pool(name='post', bufs=1) as p_post:
                # og bank (AR-independent: hides the collective)
                w_og = p_bank.tile([128, DT, HID], BF16, tag='w', bufs=2,
                                   name='w_og')
                nc.sync.dma_start(out=w_og, in_=ogw[:, :, :])
                ogs = []
                for ht in range(HT):
                    og_t = p_post.tile([128, ROWS], BF16, tag='ogs', bufs=8,
                                       name=f'ogs_{ht}')
                    for nr in range(NR):
                        sl = slice(nr * 512, (nr + 1) * 512)
                        ps = y_psum(w_og, ht, nr, f'yo_{ht}_{nr}')
                        nc.scalar.activation(og_t[:, sl], ps, SIG)
                    ogs.append(og_t)

                # q bank: psums staged to SBUF via ACT copies (AR-independent
                # so the PE never backpressures on the collective)
                w_q = p_bank.tile([128, DT, HID], BF16, tag='w', bufs=2,
                                  name='w_q')
                nc.sync.dma_start(out=w_q, in_=bank_w['q'][:, :, :])
                q_sb = []
                for ht in range(HT):
                    q_t = p_post.tile([128, ROWS], BF16, tag='q', bufs=8,
                                      name=f'q_{ht}')
                    for nr in range(NR):
                        sl = slice(nr * 512, (nr + 1) * 512)
                        ps = y_psum(w_q, ht, nr, f'yq_{ht}_{nr}')
                        nc.scalar.activation(q_t[:, sl], ps, COPY)
                    q_sb.append(q_t)

                # per-tile post-AR chain, interleaved so om tiles stream out
                # at ~2.4us each right after the collective lands:
                #   stc = cum_a * s_eff + st; out = q*stc; om = out*ogs;
                #   sq = out^2 (ACT)
                sq_tiles = []
                om_tiles = []
                for ht in range(HT):
                    cab_t = p_post.tile([128, ROWS], BF16, tag='cab', bufs=3,
                                        name=f'cab_{ht}')
                    bcast2(cab_t, ca_t, ht)
                    stc_t = p_post.tile([128, ROWS], BF16, tag='stc', bufs=3,
                                        name=f'stc_{ht}')
                    nc.vector.scalar_tensor_tensor(
                        stc_t, cab_t, s_eff[:, ht:ht + 1], st_tiles[ht],
                        MULT, ADD)
                    out_t = p_post.tile([128, ROWS], BF16, tag='out', bufs=8,
                                        name=f'out_{ht}')
                    nc.vector.tensor_mul(out_t, q_sb[ht], stc_t)
                    om_t = p_post.tile([128, ROWS], BF16, tag='om', bufs=8,
                                       name=f'om_{ht}')
                    nc.vector.tensor_mul(om_t, out_t, ogs[ht])
                    sq_t = p_post.tile([128, ROWS], BF16, tag='sq', bufs=8,
                                       name=f'sq_{ht}')
                    nc.scalar.activation(sq_t, out_t, SQUARE)
                    sq_tiles.append(sq_t)
                    om_tiles.append(om_t)

                # ---- projection nd=0 wave A: kt-outer rounds paced by om
                # production (6 banks) with sum-of-squares interleaved (2)
                WA = 6
                pjA = [p_ps.tile([128, 512], F32, tag='ps', name=f'pjA_{mr}')
                       for mr in range(WA)]
                ss_ps = [p_ps.tile([1, 512], F32, tag='ps', name=f'ss_{nr}')
                         for nr in range(NR)]
                for kt in range(HT):
                    for nr in range(NR):
                        nc.tensor.matmul(
                            ss_ps[nr], lhsT=ones_sb,
                            rhs=sq_tiles[kt][:, nr * 512:(nr + 1) * 512],
                            start=(kt == 0), stop=(kt == HT - 1))
                    for mr in range(WA):
                        nc.tensor.matmul(
                            pjA[mr],
                            lhsT=om_tiles[kt][:, mr * 128:(mr + 1) * 128],
                            rhs=opw_sb[:, kt, 0:512],
                            start=(kt == 0), stop=(kt == HT - 1))

                # rstd = sqrt(HID / (ss + HID*eps)), via DRAM-bounce
                # transpose so the reciprocal runs on 128 lanes
                ss_sb = p_res.tile([1, ROWS], F32)
                for nr in range(NR):
                    nc.vector.tensor_copy(
                        ss_sb[:, nr * 512:(nr + 1) * 512], ss_ps[nr])
                r_dram = p_dram.tile([1, ROWS], F32)
                nc.sync.dma_start(out=r_dram, in_=ss_sb)
                ss_pc = p_res.tile([128, DT], F32)
                nc.sync.dma_start(
                    out=ss_pc,
                    in_=r_dram.rearrange('one (m p) -> p (one m)', p=128))
                msb = p_res.tile([128, DT], F32)
                nc.vector.tensor_scalar_add(msb, ss_pc, HID_EPS)
                rec = p_res.tile([128, DT], F32)
                nc.vector.reciprocal(rec, msb)
                rstd_pc = p_res.tile([128, DT], F32)
                nc.scalar.activation(rstd_pc, rec, SQRT, scale=float(HID))

                def fin_store(pj, nd, mr, on_act=False):
                    fin_t = p_post.tile([128, 512], BF16, tag='fin',
                                        bufs=6, name=f'fin_{nd}_{mr}')
                    if on_act:
                        nc.scalar.activation(fin_t, pj, COPY,
                                             scale=rstd_pc[:, mr:mr + 1])
                    else:
                        nc.vector.tensor_scalar_mul(fin_t, pj,
                                                    rstd_pc[:, mr:mr + 1])
                    nc.sync.dma_start(
                        out=out_d[mr * 128:(mr + 1) * 128,
                                  nd * 512:(nd + 1) * 512],
                        in_=fin_t)

                # wave B: the remaining nd=0 row blocks (banks freed by ss),
                # then drain wave A on both engines in parallel
                for mr in range(WA, DT):
                    pj = p_ps.tile([128, 512], F32, tag='ps', name=f'pjB_{mr}')
                    for kt in range(HT):
                        nc.tensor.matmul(
                            pj,
                            lhsT=om_tiles[kt][:, mr * 128:(mr + 1) * 128],
                            rhs=opw_sb[:, kt, 0:512],
                            start=(kt == 0), stop=(kt == HT - 1))
                    fin_store(pj, 0, mr, on_act=(mr % 2 == 1))
                for mr in range(WA):
                    fin_store(pjA[mr], 0, mr, on_act=(mr % 2 == 1))

                # nd=1: mr-outer rotation (starts as wave-A banks drain)
                for mr in range(DT):
                    pj = p_ps.tile([128, 512], F32, tag='ps', name=f'pj1_{mr}')
                    for kt in range(HT):
                        nc.tensor.matmul(
                            pj,
                            lhsT=om_tiles[kt][:, mr * 128:(mr + 1) * 128],
                            rhs=opw_sb[:, kt, 512:1024],
                            start=(kt == 0), stop=(kt == HT - 1))
                    fin_store(pj, 1, mr, on_act=(mr % 2 == 1))

    nc.finalize()
    return nc


def _softmax(x):
    e = np.exp(x - x.max())
    return e / e.sum()


def _lin128(W, nt):
    """[nt*128, free] f32 -> [128, nt, free] bf16 (partition-major linear)."""
    free = W.shape[1]
    return np.ascontiguousarray(
        W.reshape(nt, 128, free).transpose(1, 0, 2)).astype(BF)


def _host_prep(inputs):
    """Build the 8 per-core input maps."""
    x = np.asarray(inputs['x'], np.float32)
    top_k = int(inputs['top_k'])

    def bank(U, V, logits):
        w = _softmax(np.asarray(logits, np.float32))
        idx = np.argsort(-w)[:top_k]
        vals = w[idx]
        vals = vals / vals.sum()
        U = np.asarray(U, np.float32)[idx]              # [k, D, R]
        V = np.asarray(V, np.float32)[idx]              # [k, R, HID]
        ucat = np.transpose(U, (1, 0, 2)).reshape(D, top_k * RANK)
        vcat = (V * vals[:, None, None]).reshape(top_k * RANK, HID)
        return _lin128(ucat @ vcat, DT)                 # [128, DT, HID]

    wg = bank(inputs['v_U'], inputs['v_V'], inputs['gate_logits'])
    wk = bank(inputs['k_U'], inputs['k_V'], inputs['k_logits'])
    wv = bank(inputs['v_U'], inputs['v_V'], inputs['v_logits'])
    wq = bank(inputs['q_U'], inputs['q_V'], inputs['q_logits'])

    ogw = _lin128(np.ascontiguousarray(
        np.asarray(inputs['out_gate_w'], np.float32).T), DT)      # [128,DT,HID]
    opw = _lin128(np.ascontiguousarray(
        (np.asarray(inputs['out_proj_w'], np.float32)
         * np.asarray(inputs['rms_w'], np.float32)[None, :]).T), HT)  # [128,HT,D]

    # decay on host (f32): z = x @ decay_w.T + b ; ld = -softplus(z)
    dw = np.asarray(inputs['decay_w'], np.float32)        # [H, D]
    db = np.asarray(inputs['decay_b'], np.float32)        # [H]
    z = np.einsum('bsd,hd->bsh', x, dw) + db              # [B, S, H]
    ld = -np.logaddexp(0.0, z)                            # log a
    a = np.exp(ld)                                        # sigmoid(-z)

    in_maps = []
    for c in range(NCORES):
        b, s2 = c // 2, c % 2
        sl = slice(s2 * ROWS, (s2 + 1) * ROWS)
        xt = _lin128(np.ascontiguousarray(x[b, sl].T), DT)          # [128,DT,ROWS]
        a_c = np.ascontiguousarray(a[b, sl].T).astype(np.float32)   # [H, ROWS]
        ca_c = np.ascontiguousarray(
            np.exp(np.cumsum(ld[b, sl], axis=0)).T).astype(BF)      # [H, ROWS]
        m_first = 1.0 if s2 == 0 else 0.0
        in_maps.append({
            'xt': xt,
            'wg': wg, 'wk': wk, 'wv': wv, 'wq': wq,
            'ogw': ogw, 'opw': opw,
            'a_t': a_c, 'ca_t': ca_c,
            'mc': np.full((128, 1), m_first, np.float32),
            'ma': np.full((128, 1), 1.0 - m_first, np.float32),
        })
    return in_maps


def kernel(**inputs) -> np.ndarray:
    from concourse.bass_utils import run_bass_kernel_spmd

    if 'nc' not in _BUILT:
        _BUILT['nc'] = _build()
    nc = _BUILT['nc']

    in_maps = _host_prep(inputs)
    res = run_bass_kernel_spmd(nc, in_maps, core_ids=list(range(NCORES)))

    out = np.empty((B, S, D), np.float32)
    for c in range(NCORES):
        b, s2 = c // 2, c % 2
        out[b, s2 * ROWS:(s2 + 1) * ROWS, :] = \
            np.asarray(res.results[c]['out'], dtype=np.float32)
    return out


# revision 19
# speedup vs baseline: 1.2520x; 1.0183x over previous
"""Trainium2 Bass kernel for CompositionalGatedRecurrence.

Strategy
--------
8 cores = (batch b, sequence-half s2).  Each core handles ROWS=1024 rows of
one batch with the FULL hidden dim, so RMSNorm and the output projection are
core-local.  The only cross-core coupling is the recurrence state at the
S/2 boundary: a [128, 8] per-pair AllReduce carries the first half's final
state to the second half.

Unlike the double-scan variant, the recurrence runs ONCE per tile with zero
init; the cross-core initial state is folded in afterwards via
    state_full[t] = state_local[t] + cumprod_a[t] * s_init
(one fused scalar_tensor_tensor per hidden tile), so the AllReduce latency
is hidden behind the two matmul banks (out_gate, q) that do not depend on it.

Engine schedule (PE FIFO order): g, k, v banks -> (AllReduce in flight)
og bank -> q bank -> sum-of-squares -> projection.  The PE never waits on
the collective.

Algebra
-------
* top-k primitive selection depends only on the logits -> done on host;
  each bank collapses to a dense W = sum_j w_j * U_j @ V_j, folded on host.
* log-decay computed on host in f32; device receives a = sigmoid(-z) = e^ld
  and its per-row running product cum_a = exp(cumsum(ld)).
* rms_w is folded into out_proj_w on host; rstd (per-row scalar) commutes
  with the hid-contraction, so it is applied AFTER the projection as a
  per-partition f32 scale on the [rows, d] psum.
* all big tensors are pre-arranged on host into [128, tile, free] layouts so
  every load is a max-rate linear DMA; x and the first bank's weights are
  chunked per dt-tile and the first bank accumulates dt-outer across 8 PSUM
  banks, so matmuls start as soon as the first chunks land.
"""

import numpy as np
import ml_dtypes

BF = ml_dtypes.bfloat16
F8NP = ml_dtypes.float8_e4m3

B, S, D = 4, 2048, 1024
H, DH = 16, 64
HID = 1024
NPRIM, RANK = 16, 256
NCORES = 8
ROWS = S // 2          # rows per core
DT = D // 128          # 8 d-model tiles
HT = HID // 128        # 8 hidden tiles
NR = ROWS // 512       # 2 row column-blocks for matmul N
EPS = float(np.finfo(np.float32).eps)
HID_EPS = float(HID * EPS)

_BUILT = {}


def _build():
    import contextlib
    import concourse.tile as tile
    from concourse import mybir, bacc

    F32 = mybir.dt.float32
    BF16 = mybir.dt.bfloat16
    MULT = mybir.AluOpType.mult
    ADD = mybir.AluOpType.add
    SIG = mybir.ActivationFunctionType.Sigmoid
    SQRT = mybir.ActivationFunctionType.Sqrt
    SQUARE = mybir.ActivationFunctionType.Square
    COPY = mybir.ActivationFunctionType.Copy

    nc = bacc.Bacc()

    F8 = mybir.dt.float8e4
    DR = mybir.MatmulPerfMode.DoubleRow

    # ---- DRAM parameters (per-core shards, host-linearized) ---------------
    xt = nc.declare_dram_parameter('xt', [128, DT, ROWS], BF16, isOutput=False)
    x8t = nc.declare_dram_parameter('x8', [128, DT, ROWS], F8, isOutput=False)
    wg8t = nc.declare_dram_parameter('wg8', [128, DT, HID], F8, isOutput=False)
    bank_w = {}
    for bk in ('k', 'v', 'q'):
        bank_w[bk] = nc.declare_dram_parameter(
            f'w{bk}', [128, DT, HID], BF16, isOutput=False)
    ogw = nc.declare_dram_parameter('ogw', [128, DT, HID], BF16, isOutput=False)
    opw = nc.declare_dram_parameter('opw', [128, HT, D], BF16, isOutput=False)
    a_t = nc.declare_dram_parameter('a_t', [H, ROWS], F32, isOutput=False)
    ca_t = nc.declare_dram_parameter('ca_t', [H, ROWS], BF16, isOutput=False)
    mc = nc.declare_dram_parameter('mc', [128, 1], F32, isOutput=False)
    ma = nc.declare_dram_parameter('ma', [128, 1], F32, isOutput=False)
    out_d = nc.declare_dram_parameter('out', [ROWS, D], BF16, isOutput=True)

    with tile.TileContext(nc, pool_alloc_mode='queue') as tc, \
            contextlib.ExitStack() as ctx:
        p_res = ctx.enter_context(tc.tile_pool(name='res', bufs=1))
        p_ps = ctx.enter_context(tc.tile_pool(name='ps', bufs=8, space='PSUM'))
        p_dram = ctx.enter_context(tc.tile_pool(name='dram', bufs=1, space='DRAM'))

        # ---- whole-kernel residents --------------------------------------
        x_sb = p_res.tile([128, DT, ROWS], BF16)
        mc_sb = p_res.tile([128, 1], F32)
        ma_sb = p_res.tile([128, 1], F32)
        ones_sb = p_res.tile([128, 1], BF16)
        st1_last = p_res.tile([128, HT], F32)   # scan final cols
        s_eff = p_res.tile([128, HT], F32)
        opw_sb = p_res.tile([128, HT, D], BF16)
        st_tiles = [p_res.tile([128, ROWS], BF16, name=f'st_{ht}')
                    for ht in range(HT)]

        nc.vector.memset(ones_sb, 1.0)

        def y_psum(w_sb, ht, nr, name):
            """Standard per-(ht,nr) dt-accumulated matmul into one psum bank."""
            ps = p_ps.tile([128, 512], F32, tag='ps', name=name)
            for dt in range(DT):
                nc.tensor.matmul(
                    ps,
                    lhsT=w_sb[:, dt, ht * 128:(ht + 1) * 128],
                    rhs=x_sb[:, dt, nr * 512:(nr + 1) * 512],
                    start=(dt == 0), stop=(dt == DT - 1))
            return ps

        def bcast2(dst, src, ht):
            """Broadcast the two head rows of tile ht to 64 lanes each."""
            nc.sync.dma_start(
                out=dst[0:64, :],
                in_=src[2 * ht:2 * ht + 1, :].to_broadcast([64, ROWS]))
            nc.sync.dma_start(
                out=dst[64:128, :],
                in_=src[2 * ht + 1:2 * ht + 2, :].to_broadcast([64, ROWS]))

        # =========== phase 1: banks g, k, v with progressive kv fuse ======
        with tc.tile_pool(name='bank', bufs=1) as p_bank:
            with tc.tile_pool(name='fuse', bufs=1) as p_fuse, \
                    tc.tile_pool(name='abp', bufs=1) as p_ab:
                # g bank: fp8 DoubleRow (sigmoid compresses the quantization
                # noise), chunked DMA + dt-outer accumulation for early start
                wg8 = p_fuse.tile([128, DT, HID], F8, name='wg8')
                x8_sb = p_fuse.tile([128, DT, ROWS], F8, name='x8')
                for dt in range(DT):
                    nc.sync.dma_start(out=wg8[:, dt, :], in_=wg8t[:, dt, :])
                    nc.sync.dma_start(out=x8_sb[:, dt, :], in_=x8t[:, dt, :])
                    nc.sync.dma_start(out=x_sb[:, dt, :], in_=xt[:, dt, :])
                nc.sync.dma_start(out=mc_sb, in_=mc[:, :])
                nc.sync.dma_start(out=ma_sb, in_=ma[:, :])

                sigg = [p_fuse.tile([128, ROWS], BF16, tag='sigg', bufs=8,
                                    name=f'sigg_{ht}') for ht in range(HT)]
                for nr in range(NR):
                    pss = [p_ps.tile([128, 512], F32, tag='ps',
                                     name=f'yg_{ht}_{nr}') for ht in range(HT)]
                    for dt in range(0, DT, 2):
                        for ht in range(HT):
                            nc.tensor.matmul(
                                pss[ht],
                                lhsT=wg8[:, dt:dt + 2, ht * 128:(ht + 1) * 128],
                                rhs=x8_sb[:, dt:dt + 2,
                                          nr * 512:(nr + 1) * 512],
                                start=(dt == 0), stop=(dt == DT - 2),
                                perf_mode=DR)
                    for ht in range(HT):
                        nc.scalar.activation(
                            sigg[ht][:, nr * 512:(nr + 1) * 512], pss[ht], SIG)

                # k bank
                w_k = p_bank.tile([128, DT, HID], BF16, tag='w', bufs=2,
                                  name='w_k')
                nc.sync.dma_start(out=w_k, in_=bank_w['k'][:, :, :])
                tgk = []
                for ht in range(HT):
                    tk_t = p_fuse.tile([128, ROWS], BF16, tag='tgk', bufs=8,
                                       name=f'tgk_{ht}')
                    for nr in range(NR):
                        sl = slice(nr * 512, (nr + 1) * 512)
                        ps = y_psum(w_k, ht, nr, f'yk_{ht}_{nr}')
                        nc.vector.tensor_mul(tk_t[:, sl], ps, sigg[ht][:, sl])
                    tgk.append(tk_t)

                # v bank + scan (zero-init local states)
                w_v = p_bank.tile([128, DT, HID], BF16, tag='w', bufs=2,
                                  name='w_v')
                nc.sync.dma_start(out=w_v, in_=bank_w['v'][:, :, :])
                # prefetch the projection weights while PE is busy here
                nc.sync.dma_start(out=opw_sb,
                                  in_=opw[:, :, :])
                for ht in range(HT):
                    kv_t = p_fuse.tile([128, ROWS], F32, tag='kv', bufs=3,
                                       name=f'kv_{ht}')
                    for nr in range(NR):
                        sl = slice(nr * 512, (nr + 1) * 512)
                        ps = y_psum(w_v, ht, nr, f'yv_{ht}_{nr}')
                        nc.vector.tensor_mul(kv_t[:, sl], ps, tgk[ht][:, sl])
                    ab_t = p_ab.tile([128, ROWS], F32, tag='ab', bufs=2,
                                     name=f'ab_{ht}')
                    bcast2(ab_t, a_t, ht)
                    nc.vector.tensor_tensor_scan(
                        st_tiles[ht], ab_t, kv_t, 0.0, MULT, ADD)
                    nc.vector.tensor_copy(st1_last[:, ht:ht + 1],
                                          st_tiles[ht][:, ROWS - 1:ROWS])

                # ---- boundary state exchange (pairs) ---------------------
                contrib = p_res.tile([128, HT], F32)
                nc.vector.tensor_scalar_mul(contrib, st1_last, mc_sb)
                cin = p_dram.tile([128, HT], F32)
                cout = p_dram.tile([128, HT], F32)
                nc.sync.dma_start(out=cin, in_=contrib)
                nc.gpsimd.collective_compute(
                    'AllReduce', ADD,
                    replica_groups=[[0, 1], [2, 3], [4, 5], [6, 7]],
                    ins=[cin.opt()], outs=[cout.opt()])
                s_init = p_res.tile([128, HT], F32)
                nc.sync.dma_start(out=s_init, in_=cout)
                nc.vector.tensor_scalar_mul(s_eff, s_init, ma_sb)
            # p_fuse / p_ab closed: sigg, tgk, kv, ab freed

            # =========== phase 2: og bank, correction, q bank =============
            with tc.tile_pool(name='post', bufs=1) as p_post:
                # og bank (AR-independent: hides the collective)
                w_og = p_bank.tile([128, DT, HID], BF16, tag='w', bufs=2,
                                   name='w_og')
                nc.sync.dma_start(out=w_og, in_=ogw[:, :, :])
                ogs = []
                for ht in range(HT):
                    og_t = p_post.tile([128, ROWS], BF16, tag='ogs', bufs=8,
                                       name=f'ogs_{ht}')
                    for nr in range(NR):
                        sl = slice(nr * 512, (nr + 1) * 512)
                        ps = y_psum(w_og, ht, nr, f'yo_{ht}_{nr}')
                        nc.scalar.activation(og_t[:, sl], ps, SIG)
                    ogs.append(og_t)

                # q bank: psums staged to SBUF via ACT copies (AR-independent
                # so the PE never backpressures on the collective)
                w_q = p_bank.tile([128, DT, HID], BF16, tag='w', bufs=2,
                                  name='w_q')
                nc.sync.dma_start(out=w_q, in_=bank_w['q'][:, :, :])
                q_sb = []
                for ht in range(HT):
                    q_t = p_post.tile([128, ROWS], BF16, tag='q', bufs=8,
                                      name=f'q_{ht}')
                    for nr in range(NR):
                        sl = slice(nr * 512, (nr + 1) * 512)
                        ps = y_psum(w_q, ht, nr, f'yq_{ht}_{nr}')
                        nc.scalar.activation(q_t[:, sl], ps, COPY)
                    q_sb.append(q_t)

                # per-tile post-AR chain, interleaved so om tiles stream out
                # at ~2.4us each right after the collective lands:
                #   stc = cum_a * s_eff + st; out = q*stc; om = out*ogs;
                #   sq = out^2 (ACT)
                sq_tiles = []
                om_tiles = []
                for ht in range(HT):
                    cab_t = p_post.tile([128, ROWS], BF16, tag='cab', bufs=3,
                                        name=f'cab_{ht}')
                    bcast2(cab_t, ca_t, ht)
                    stc_t = p_post.tile([128, ROWS], BF16, tag='stc', bufs=3,
                                        name=f'stc_{ht}')
                    nc.vector.scalar_tensor_tensor(
                        stc_t, cab_t, s_eff[:, ht:ht + 1], st_tiles[ht],
                        MULT, ADD)
                    out_t = p_post.tile([128, ROWS], BF16, tag='out', bufs=8,
                                        name=f'out_{ht}')
                    nc.vector.tensor_mul(out_t, q_sb[ht], stc_t)
                    om_t = p_post.tile([128, ROWS], BF16, tag='om', bufs=8,
                                       name=f'om_{ht}')
                    nc.vector.tensor_mul(om_t, out_t, ogs[ht])
                    sq_t = p_post.tile([128, ROWS], BF16, tag='sq', bufs=8,
                                       name=f'sq_{ht}')
                    nc.scalar.activation(sq_t, out_t, SQUARE)
                    sq_tiles.append(sq_t)
                    om_tiles.append(om_t)

                # sum-of-squares right after the q bank (paced by the ACT
                # squares) so rstd is ready before the projection needs it
                ss_ps = [p_ps.tile([1, 512], F32, tag='ps', name=f'ss_{nr}')
                         for nr in range(NR)]
                for kt in range(HT):
                    for nr in range(NR):
                        nc.tensor.matmul(
                            ss_ps[nr], lhsT=ones_sb,
                            rhs=sq_tiles[kt][:, nr * 512:(nr + 1) * 512],
                            start=(kt == 0), stop=(kt == HT - 1))

                # rstd = sqrt(HID / (ss + HID*eps)), via DRAM-bounce
                # transpose so the reciprocal runs on 128 lanes
                ss_sb = p_res.tile([1, ROWS], F32)
                for nr in range(NR):
                    nc.vector.tensor_copy(
                        ss_sb[:, nr * 512:(nr + 1) * 512], ss_ps[nr])
                r_dram = p_dram.tile([1, ROWS], F32)
                nc.sync.dma_start(out=r_dram, in_=ss_sb)
                ss_pc = p_res.tile([128, DT], F32)
                nc.sync.dma_start(
                    out=ss_pc,
                    in_=r_dram.rearrange('one (m p) -> p (one m)', p=128))
                msb = p_res.tile([128, DT], F32)
                nc.vector.tensor_scalar_add(msb, ss_pc, HID_EPS)
                rec = p_res.tile([128, DT], F32)
                nc.vector.reciprocal(rec, msb)
                rstd_pc = p_res.tile([128, DT], F32)
                nc.scalar.activation(rstd_pc, rec, SQRT, scale=float(HID))

                def fin_store(pj, nd, mr, on_act=False):
                    fin_t = p_post.tile([128, 512], BF16, tag='fin',
                                        bufs=6, name=f'fin_{nd}_{mr}')
                    if on_act:
                        nc.scalar.activation(fin_t, pj, COPY,
                                             scale=rstd_pc[:, mr:mr + 1])
                    else:
                        nc.vector.tensor_scalar_mul(fin_t, pj,
                                                    rstd_pc[:, mr:mr + 1])
                    nc.sync.dma_start(
                        out=out_d[mr * 128:(mr + 1) * 128,
                                  nd * 512:(nd + 1) * 512],
                        in_=fin_t)

                # ---- projection: two mr-outer passes, fins drain on both
                # engines while the PE streams the next group
                for nd in range(2):
                    for mr in range(DT):
                        pj = p_ps.tile([128, 512], F32, tag='ps',
                                       name=f'pj{nd}_{mr}')
                        for kt in range(HT):
                            nc.tensor.matmul(
                                pj,
                                lhsT=om_tiles[kt][:, mr * 128:(mr + 1) * 128],
                                rhs=opw_sb[:, kt, nd * 512:(nd + 1) * 512],
                                start=(kt == 0), stop=(kt == HT - 1))
                        fin_store(pj, nd, mr, on_act=(mr % 2 == 1))

    nc.finalize()
    return nc


def _softmax(x):
    e = np.exp(x - x.max())
    return e / e.sum()


def _lin128(W, nt, dtype=BF):
    """[nt*128, free] f32 -> [128, nt, free] (partition-major linear)."""
    free = W.shape[1]
    return np.ascontiguousarray(
        W.reshape(nt, 128, free).transpose(1, 0, 2)).astype(dtype)


def _host_prep(inputs):
    """Build the 8 per-core input maps."""
    x = np.asarray(inputs['x'], np.float32)
    top_k = int(inputs['top_k'])

    def bank(U, V, logits, dtype=BF):
        w = _softmax(np.asarray(logits, np.float32))
        idx = np.argsort(-w)[:top_k]
        vals = w[idx]
        vals = vals / vals.sum()
        U = np.asarray(U, np.float32)[idx]              # [k, D, R]
        V = np.asarray(V, np.float32)[idx]              # [k, R, HID]
        ucat = np.transpose(U, (1, 0, 2)).reshape(D, top_k * RANK)
        vcat = (V * vals[:, None, None]).reshape(top_k * RANK, HID)
        return _lin128(ucat @ vcat, DT, dtype)          # [128, DT, HID]

    wg8 = bank(inputs['v_U'], inputs['v_V'], inputs['gate_logits'], F8NP)
    wk = bank(inputs['k_U'], inputs['k_V'], inputs['k_logits'])
    wv = bank(inputs['v_U'], inputs['v_V'], inputs['v_logits'])
    wq = bank(inputs['q_U'], inputs['q_V'], inputs['q_logits'])

    ogw = _lin128(np.ascontiguousarray(
        np.asarray(inputs['out_gate_w'], np.float32).T), DT)      # [128,DT,HID]
    opw = _lin128(np.ascontiguousarray(
        (np.asarray(inputs['out_proj_w'], np.float32)
         * np.asarray(inputs['rms_w'], np.float32)[None, :]).T), HT)  # [128,HT,D]

    # decay on host (f32): z = x @ decay_w.T + b ; ld = -softplus(z)
    dw = np.asarray(inputs['decay_w'], np.float32)        # [H, D]
    db = np.asarray(inputs['decay_b'], np.float32)        # [H]
    z = np.einsum('bsd,hd->bsh', x, dw) + db              # [B, S, H]
    ld = -np.logaddexp(0.0, z)                            # log a
    a = np.exp(ld)                                        # sigmoid(-z)

    in_maps = []
    for c in range(NCORES):
        b, s2 = c // 2, c % 2
        sl = slice(s2 * ROWS, (s2 + 1) * ROWS)
        xc = np.ascontiguousarray(x[b, sl].T)
        xt = _lin128(xc, DT)                                        # [128,DT,ROWS]
        x8 = _lin128(xc, DT, F8NP)
        a_c = np.ascontiguousarray(a[b, sl].T).astype(np.float32)   # [H, ROWS]
        ca_c = np.ascontiguousarray(
            np.exp(np.cumsum(ld[b, sl], axis=0)).T).astype(BF)      # [H, ROWS]
        m_first = 1.0 if s2 == 0 else 0.0
        in_maps.append({
            'xt': xt, 'x8': x8, 'wg8': wg8,
            'wk': wk, 'wv': wv, 'wq': wq,
            'ogw': ogw, 'opw': opw,
            'a_t': a_c, 'ca_t': ca_c,
            'mc': np.full((128, 1), m_first, np.float32),
            'ma': np.full((128, 1), 1.0 - m_first, np.float32),
        })
    return in_maps


def kernel(**inputs) -> np.ndarray:
    from concourse.bass_utils import run_bass_kernel_spmd

    if 'nc' not in _BUILT:
        _BUILT['nc'] = _build()
    nc = _BUILT['nc']

    in_maps = _host_prep(inputs)
    res = run_bass_kernel_spmd(nc, in_maps, core_ids=list(range(NCORES)))

    out = np.empty((B, S, D), np.float32)
    for c in range(NCORES):
        b, s2 = c // 2, c % 2
        out[b, s2 * ROWS:(s2 + 1) * ROWS, :] = \
            np.asarray(res.results[c]['out'], dtype=np.float32)
    return out
